# revision 9
# baseline (speedup 1.0000x reference)
"""Trainium2 kernel v3 for nn_Loss_26886495273741 (retrieval_knn).

reference:
    dots = feature @ feature.T          # [n, n], n=16384, d=256
    dots[diag] = -1
    I = argmax(dots, axis=1)
    loss = -mean(log(n * ||feature - feature[I] + 1e-6||_2))

Device strategy (8 NeuronCores, SPMD, host-replicated "all-gather"):
  * Rows sharded: core c owns rows [c*2048, (c+1)*2048).
  * fp8 DoubleRow matmuls fill PSUM tiles [128, 1024] fp32 (x4 bufs)
    with the row-block of dots, 16 tiles per 128-row tile.
  * Hardware constraints (verified): only ACT and DVE can read PSUM and
    only one PSUM operand per instruction, so the drain is split:
      V: DVE absorb  tensor_tensor max(ps, acc_v) -> acc_v   (bf16 acc)
      A: ACT copy -> bf16 s;  Pool folds s into acc_p
      B: ACT copy -> bf16 s;  DVE folds s into acc_v
      L: ACT exp(s*dot) with fp32 accumulator -> per-chunk LSE score
         (self-contained; those columns are covered by chunk scores,
         not by the class fold)
  * acc_v/acc_p are halved to W classes (DVE / Pool) and shipped; LSE
    chunk scores ship as fp32. Host: top-K classes + top LSE chunks +
    diagonal class/chunk, exact fp32 candidate eval, reference loss.
"""

import os
import sys

import numpy as np

_jp = os.environ.get("JAX_PLATFORMS")
if _jp is not None and "axon" not in _jp:
    os.environ["JAX_PLATFORMS"] = "axon," + _jp

try:
    import concourse.bass as bass  # noqa: F401
except ImportError:  # grading env runs from a bare directory
    sys.path.insert(0, "/opt/trn_rl_repo")

import ml_dtypes  # noqa: F401

import concourse.bass as bass
import concourse.mybir as mybir
import concourse.tile as tile
from concourse import bacc
from concourse.bass_utils import run_bass_kernel_spmd

# Problem geometry (hardcoded per spec.json: feature [16384, 256] f32).
N = 16384
D = 256
N_CORES = 8
ROWS_PER_CORE = N // N_CORES  # 2048
P = 128
ROW_TILES = ROWS_PER_CORE // P  # 16
CHUNK = 512  # matmul free dim == one PSUM bank (fp32)
KH = D // P  # 2 contraction halves packed for DoubleRow

EPS = 1e-6

_BF16 = mybir.dt.bfloat16
_F32 = mybir.dt.float32
_FP8 = mybir.dt.float8e4
_FP8_NP = mybir.dt.np(_FP8)

_MAX = mybir.AluOpType.max
_ADD = mybir.AluOpType.add

# --- tunables (swept offline with TimelineSim; best kept hardcoded) ---
PSW = 1024  # psum tile width (2 banks, 4 bufs)
# per-row-tile channel patterns, cycled over row tiles.
# V=DVE absorb, A=ACT copy+Pool fold, B=ACT copy+DVE fold, L=ACT LSE
PATTERNS = ("VGVLVLVGVLVLVLVL", "VLVGVLVLVLVLVLVL")
W_SHIP = 1024  # classes shipped per row (= PSW: no halving)
FT_SPLIT = 12  # column blocks for the big ft load
TOPK = 4  # host-side top-k classes
TOPC = 3  # host-side top-k LSE chunks
LSE_SCALE = 0.275  # exp scale for LSE scores

NEG = -3.0e38


def build_nc(
    psw: int = PSW,
    patterns: tuple = PATTERNS,
    w_ship: int = W_SHIP,
    ft_split: int = FT_SPLIT,
    psum_bufs: int = 0,
):
    n_ps = N // psw  # psum tiles per row-tile
    chunks_per_ps = psw // CHUNK
    if not psum_bufs:
        psum_bufs = (16 * 1024) // (psw * 4)  # fill all 8 banks
    n_lse = max(pat.count("L") for pat in patterns)
    n_g = max(pat.count("G") for pat in patterns)
    for pat in patterns:
        assert len(pat) == n_ps, pat
        assert all(c in "VABLG" for c in pat)
        assert pat[0] == "V", "first V initializes acc_v"

    nc = bacc.Bacc("TRN2", target_bir_lowering=False, debug=False)

    ft_dram = nc.dram_tensor("ft", [P, KH, N], _FP8, kind="ExternalInput")
    at_dram = nc.dram_tensor(
        "at", [P, KH, ROWS_PER_CORE], _FP8, kind="ExternalInput"
    )
    cls_dram = nc.dram_tensor(
        "cls", [ROW_TILES, P, 2, w_ship], _BF16, kind="ExternalOutput"
    )
    lse_dram = nc.dram_tensor(
        "lse", [ROW_TILES, P, max(n_lse, 1)], _F32, kind="ExternalOutput"
    )
    gcl_dram = nc.dram_tensor(
        "gcl", [ROW_TILES, P, 2, w_ship if n_g else 1], _BF16,
        kind="ExternalOutput",
    )

    with tile.TileContext(nc) as tc:
        with (
            tc.tile_pool(name="ft_pool", bufs=1) as ft_pool,
            tc.tile_pool(name="at_pool", bufs=1) as at_pool,
            tc.tile_pool(name="s_pool", bufs=8) as s_pool,
            tc.tile_pool(name="d_pool", bufs=4) as d_pool,
            tc.tile_pool(name="acc_pool", bufs=8) as acc_pool,
            tc.tile_pool(name="sc_pool", bufs=2) as sc_pool,
            tc.tile_pool(name="psum", bufs=psum_bufs, space="PSUM") as psum_pool,
        ):
            at_sb = at_pool.tile([P, KH, ROWS_PER_CORE], _FP8, tag="at")
            # first row-tile's weights first, then the rest
            nc.sync.dma_start(at_sb[:, :, 0:P], at_dram[:, :, 0:P])
            ft_sb = ft_pool.tile([P, KH, N], _FP8, tag="ft")
            # small first blocks so the first matmuls can start early
            cuts = [0, 1024, 2048, 4096]
            blk = (N - 4096) // ft_split
            cuts += [4096 + j * blk for j in range(1, ft_split)] + [N]
            for j0, j1 in zip(cuts, cuts[1:]):
                nc.sync.dma_start(ft_sb[:, :, j0:j1], ft_dram[:, :, j0:j1])
                if j0 == 0:
                    nc.sync.dma_start(
                        at_sb[:, :, P:ROWS_PER_CORE],
                        at_dram[:, :, P:ROWS_PER_CORE],
                    )

            dummies = [
                d_pool.tile([P, psw], _BF16, name=f"dummy{i}", tag="dummy")
                for i in range(4)
            ]


            for r in range(ROW_TILES):
                pattern = patterns[r % len(patterns)]
                accs = [
                    acc_pool.tile([P, psw], _BF16, name=f"acc{i}", tag="accv")
                    for i in range(2)
                ]
                scores = sc_pool.tile([P, max(n_lse, 1)], _F32, tag="sc")
                seen = [False, False]
                vi = 0
                li = 0
                acc_g = None
                gi = 0
                if pattern.count("G"):
                    acc_g = [
                        acc_pool.tile([P, psw], _BF16, name=f"accg{i}", tag="accg")
                        for i in range(2)
                    ]
                for t in range(n_ps):
                    ps = psum_pool.tile([P, psw], _F32, tag="ps")
                    for h in range(chunks_per_ps):
                        c = (t * psw) // CHUNK + h
                        nc.tensor.matmul(
                            ps[:, h * CHUNK : (h + 1) * CHUNK],
                            at_sb[:, :, r * P : (r + 1) * P],
                            ft_sb[:, :, c * CHUNK : (c + 1) * CHUNK],
                            start=True,
                            stop=True,
                            perf_mode=mybir.MatmulPerfMode.DoubleRow,
                        )
                    ch = pattern[t]
                    if ch == "V":
                        idx = vi % 2
                        a = accs[idx]
                        vi += 1
                        if not seen[idx]:
                            nc.vector.tensor_scalar_max(a[:], ps[:], NEG)
                            seen[idx] = True
                        else:
                            nc.vector.tensor_tensor(a[:], ps[:], a[:], _MAX)
                    elif ch in "AB":
                        s = s_pool.tile([P, psw], _BF16, tag="sb")
                        nc.scalar.copy(s[:], ps[:])
                        idx = vi % 2
                        a = accs[idx]
                        vi += 1
                        if not seen[idx]:
                            raise AssertionError("fold before acc init")
                        nc.vector.tensor_tensor(a[:], s[:], a[:], _MAX)
                    elif ch == "L":
                        nc.scalar.activation(
                            dummies[li % 2][:],
                            ps[:],
                            mybir.ActivationFunctionType.Exp,
                            scale=LSE_SCALE,
                            accum_out=scores[:, li : li + 1],
                        )
                        li += 1
                    elif ch == "G":
                        a_g = acc_g[gi % 2]
                        if gi < 2:
                            # first two G copies initialize the G-accs
                            nc.scalar.activation(
                                a_g[:],
                                ps[:],
                                mybir.ActivationFunctionType.Exp,
                                scale=LSE_SCALE,
                            )
                        else:
                            sg = s_pool.tile([P, psw], _BF16, tag="sg")
                            nc.scalar.activation(
                                sg[:],
                                ps[:],
                                mybir.ActivationFunctionType.Exp,
                                scale=LSE_SCALE,
                            )
                            nc.gpsimd.tensor_tensor(
                                a_g[:], sg[:], a_g[:], _ADD
                            )
                        gi += 1
                # ship both raw accumulators; host maxes them
                assert seen[0] and seen[1] and w_ship == psw
                nc.sync.dma_start(cls_dram[r, :, 0, :], accs[0][:])
                nc.sync.dma_start(cls_dram[r, :, 1, :], accs[1][:])
                if acc_g is not None:
                    nc.sync.dma_start(gcl_dram[r, :, 0, :], acc_g[0][:])
                    if gi > 1:
                        nc.sync.dma_start(gcl_dram[r, :, 1, :], acc_g[1][:])
                if n_lse:
                    nc.sync.dma_start(lse_dram[r], scores[:])

    nc.compile()
    return nc


_NC_CACHE = {}


def _get_nc():
    if "nc" not in _NC_CACHE:
        _NC_CACHE["nc"] = build_nc()
    return _NC_CACHE["nc"]


def make_inputs(feature: np.ndarray):
    """Host-side shard prep: F^T in [P, KH, cols] layout, fp8-quantized."""
    ft = np.ascontiguousarray(
        feature.T.reshape(KH, P, N).transpose(1, 0, 2)
    ).astype(_FP8_NP)
    in_maps = []
    for c in range(N_CORES):
        at = np.ascontiguousarray(
            ft[:, :, c * ROWS_PER_CORE : (c + 1) * ROWS_PER_CORE]
        )
        in_maps.append({"ft": ft, "at": at})
    return in_maps


def run_device(feature: np.ndarray, trace: bool = False):
    """Run the SPMD kernel.

    Returns (cls [N, W] f32 class maxima over non-L columns,
             lse [N, n_lse] f32 chunk scores, results)."""
    nc = _get_nc()
    in_maps = make_inputs(feature)
    res = run_bass_kernel_spmd(
        nc, in_maps, core_ids=list(range(N_CORES)), trace=trace
    )
    cls_parts, lse_parts, g_parts = [], [], []
    for r in res.results:
        c = np.asarray(r["cls"]).astype(np.float32)  # [RT, P, 2, W]
        cls_parts.append(c.max(axis=2).reshape(ROWS_PER_CORE, -1))
        lse_parts.append(np.asarray(r["lse"]).reshape(ROWS_PER_CORE, -1))
        g = np.asarray(r["gcl"]).astype(np.float32)  # [RT, P, 2, W]
        g_parts.append(g.sum(axis=2).reshape(ROWS_PER_CORE, -1))
    return (
        np.concatenate(cls_parts),
        np.concatenate(lse_parts),
        np.concatenate(g_parts),
        res,
    )


def recover_loss(
    feature: np.ndarray,
    cls: np.ndarray,
    lse: np.ndarray,
    gcl: np.ndarray | None = None,
) -> np.float32:
    """Exact argmax recovery + reference loss formula on host.

    cls[g] = per-class maxima (class = col mod W) over columns drained
    through the fold channels; lse[g] = per-L-chunk exp-sum scores for
    columns covered by LSE tiles. Candidates per row: top-K classes,
    the diagonal class, top-C LSE chunks, and the diagonal's own chunk
    if it lies in an LSE tile. All candidates evaluated in exact fp32.
    """
    n, w = feature.shape[0], cls.shape[1]
    B = n // w
    psw = PSW
    feat = np.ascontiguousarray(feature, dtype=np.float32)
    rows = np.arange(n)
    k = min(TOPK, w - 1)
    t_cls = np.argpartition(-cls, k, axis=1)[:, :k].astype(np.int64)

    best_val = np.full(n, -np.inf, dtype=np.float32)
    best_col = np.zeros(n, dtype=np.int64)

    def consider_cols(row_idx: np.ndarray, cols: np.ndarray):
        cd = feat[row_idx] @ feat[cols].T
        self_pos = np.searchsorted(cols, row_idx)
        kk = np.arange(len(row_idx))
        has_self = (self_pos < len(cols)) & (
            cols[np.minimum(self_pos, len(cols) - 1)] == row_idx
        )
        cd[kk[has_self], self_pos[has_self]] = -np.inf
        b = np.argmax(cd, axis=1)
        v = cd[kk, b]
        c = cols[b]
        upd = (v > best_val[row_idx]) | (
            (v == best_val[row_idx]) & (c < best_col[row_idx])
        )
        ri = row_idx[upd]
        best_val[ri] = v[upd]
        best_col[ri] = c[upd]

    # class candidates: top-K device classes (+ exp-domain G classes)
    # + diagonal class
    parts = [t_cls]
    if gcl is not None and gcl.shape[1] == w:
        kg = min(3, w - 1)
        parts.append(
            np.argpartition(-np.nan_to_num(gcl, nan=-np.inf), kg, axis=1)[
                :, :kg
            ].astype(np.int64)
        )
    parts.append((rows % w)[:, None])
    all_cls = np.concatenate(parts, axis=1)
    for j in range(all_cls.shape[1]):
        col = all_cls[:, j]
        order = np.argsort(col, kind="stable")
        bounds = np.searchsorted(col[order], np.arange(w + 1))
        for t in range(w):
            grp = order[bounds[t] : bounds[t + 1]]
            if len(grp):
                consider_cols(grp, t + w * np.arange(B))

    # LSE chunk candidates
    n_ps = N // psw
    lpos = {}  # row-tile index -> list of L tile positions
    for r in range(ROW_TILES):
        pat = PATTERNS[r % len(PATTERNS)]
        lpos[r] = [t for t in range(n_ps) if pat[t] == "L"]
    n_lse = lse.shape[1]
    if n_lse:
        rt = (rows % ROWS_PER_CORE) // P  # row-tile index per row
        lchunks = np.full((n, n_lse), -1, dtype=np.int64)
        for r in range(ROW_TILES):
            sel = rt == r
            for j, t in enumerate(lpos[r]):
                lchunks[sel, j] = t
        # slots without an L tile carry garbage scores; mask them out
        lse = np.where(
            lchunks >= 0, np.nan_to_num(lse, nan=-np.inf), -np.inf
        )
        cc = min(TOPC, n_lse)
        top = np.argpartition(-lse, cc - 1, axis=1)[:, :cc]
        want = np.zeros((n, n_ps), dtype=bool)
        kk = np.arange(n)[:, None]
        want[kk, lchunks[kk, top]] = True
        # diagonal chunk if the row's own column lies in an L tile
        diag_t = rows // psw % n_ps
        in_l = lchunks == diag_t[:, None]
        want[in_l.any(axis=1), diag_t[in_l.any(axis=1)]] = True
        for t in range(n_ps):
            grp = rows[want[:, t]]
            if len(grp):
                consider_cols(grp, np.arange(t * psw, (t + 1) * psw))

    I = best_col
    diff = feat - feat[I] + EPS
    dist = np.sqrt((diff * diff).sum(axis=1))
    loss = -np.mean(np.log(n * dist))
    return np.float32(loss)


def kernel(feature: np.ndarray) -> np.ndarray:
    feature = np.asarray(feature, dtype=np.float32)
    for attempt in range(3):
        try:
            cls, lse, gcl, _res = run_device(feature)
            break
        except Exception:
            # transient device/tunnel hiccups; rebuild and retry
            _NC_CACHE.clear()
            if attempt == 2:
                raise
    return np.asarray(recover_loss(feature, cls, lse, gcl), dtype=np.float32)


if __name__ == "__main__":
    rng = np.random.default_rng(0)
    feature = rng.standard_normal((N, D), dtype=np.float32)
    print("loss:", kernel(feature))


# revision 10
# speedup vs baseline: 1.0506x; 1.0506x over previous
"""Trainium2 kernel v3 for nn_Loss_26886495273741 (retrieval_knn).

reference:
    dots = feature @ feature.T          # [n, n], n=16384, d=256
    dots[diag] = -1
    I = argmax(dots, axis=1)
    loss = -mean(log(n * ||feature - feature[I] + 1e-6||_2))

Device strategy (8 NeuronCores, SPMD, host-replicated "all-gather"):
  * Rows sharded: core c owns rows [c*2048, (c+1)*2048).
  * fp8 DoubleRow matmuls fill PSUM tiles [128, 1024] fp32 (x4 bufs)
    with the row-block of dots, 16 tiles per 128-row tile.
  * Hardware constraints (verified): only ACT and DVE can read PSUM and
    only one PSUM operand per instruction, so the drain is split:
      V: DVE absorb  tensor_tensor max(ps, acc_v) -> acc_v   (bf16 acc)
      A: ACT copy -> bf16 s;  Pool folds s into acc_p
      B: ACT copy -> bf16 s;  DVE folds s into acc_v
      L: ACT exp(s*dot) with fp32 accumulator -> per-chunk LSE score
         (self-contained; those columns are covered by chunk scores,
         not by the class fold)
  * acc_v/acc_p are halved to W classes (DVE / Pool) and shipped; LSE
    chunk scores ship as fp32. Host: top-K classes + top LSE chunks +
    diagonal class/chunk, exact fp32 candidate eval, reference loss.
"""

import os
import sys

import numpy as np

_jp = os.environ.get("JAX_PLATFORMS")
if _jp is not None and "axon" not in _jp:
    os.environ["JAX_PLATFORMS"] = "axon," + _jp

try:
    import concourse.bass as bass  # noqa: F401
except ImportError:  # grading env runs from a bare directory
    sys.path.insert(0, "/opt/trn_rl_repo")

import ml_dtypes  # noqa: F401

import concourse.bass as bass
import concourse.mybir as mybir
import concourse.tile as tile
from concourse import bacc
from concourse.bass_utils import run_bass_kernel_spmd

# Problem geometry (hardcoded per spec.json: feature [16384, 256] f32).
N = 16384
D = 256
N_CORES = 8
ROWS_PER_CORE = N // N_CORES  # 2048
P = 128
ROW_TILES = ROWS_PER_CORE // P  # 16
CHUNK = 512  # matmul free dim == one PSUM bank (fp32)
KH = D // P  # 2 contraction halves packed for DoubleRow

EPS = 1e-6

_BF16 = mybir.dt.bfloat16
_F32 = mybir.dt.float32
_FP8 = mybir.dt.float8e4
_FP8_NP = mybir.dt.np(_FP8)

_MAX = mybir.AluOpType.max
_ADD = mybir.AluOpType.add

# --- tunables (swept offline with TimelineSim; best kept hardcoded) ---
PSW = 1024  # psum tile width (2 banks, 4 bufs)
# per-row-tile channel patterns, cycled over row tiles.
# V=DVE absorb, A=ACT copy+Pool fold, B=ACT copy+DVE fold, L=ACT LSE
PATTERNS = ("VGVLVLVGVLVLVLVL", "VLVGVLVLVLVLVLVL")
W_SHIP = 1024  # classes shipped per row (= PSW: no halving)
FT_SPLIT = 12  # column blocks for the big ft load
TOPK = 4  # host-side top-k classes
TOPC = 3  # host-side top-k LSE chunks
LSE_SCALE = 0.275  # exp scale for LSE scores

NEG = -3.0e38


def build_nc(
    psw: int = PSW,
    patterns: tuple = PATTERNS,
    w_ship: int = W_SHIP,
    ft_split: int = FT_SPLIT,
    psum_bufs: int = 0,
):
    n_ps = N // psw  # psum tiles per row-tile
    chunks_per_ps = psw // CHUNK
    if not psum_bufs:
        psum_bufs = (16 * 1024) // (psw * 4)  # fill all 8 banks
    n_lse = max(pat.count("L") for pat in patterns)
    n_g = max(pat.count("G") for pat in patterns)
    for pat in patterns:
        assert len(pat) == n_ps, pat
        assert all(c in "VABLG" for c in pat)
        assert pat[0] == "V", "first V initializes acc_v"

    nc = bacc.Bacc("TRN2", target_bir_lowering=False, debug=False)

    ft_dram = nc.dram_tensor("ft", [P, KH, N], _FP8, kind="ExternalInput")
    at_dram = nc.dram_tensor(
        "at", [P, KH, ROWS_PER_CORE], _FP8, kind="ExternalInput"
    )
    cls_dram = nc.dram_tensor(
        "cls", [ROW_TILES, P, 2, w_ship], _BF16, kind="ExternalOutput"
    )
    lse_dram = nc.dram_tensor(
        "lse", [ROW_TILES, P, max(n_lse, 1)], _F32, kind="ExternalOutput"
    )
    gcl_dram = nc.dram_tensor(
        "gcl", [ROW_TILES, P, 2, w_ship if n_g else 1], _BF16,
        kind="ExternalOutput",
    )

    with tile.TileContext(nc) as tc:
        with (
            tc.tile_pool(name="ft_pool", bufs=1) as ft_pool,
            tc.tile_pool(name="at_pool", bufs=1) as at_pool,
            tc.tile_pool(name="s_pool", bufs=8) as s_pool,
            tc.tile_pool(name="d_pool", bufs=4) as d_pool,
            tc.tile_pool(name="acc_pool", bufs=8) as acc_pool,
            tc.tile_pool(name="sc_pool", bufs=2) as sc_pool,
            tc.tile_pool(name="psum", bufs=psum_bufs, space="PSUM") as psum_pool,
        ):
            at_sb = at_pool.tile([P, KH, ROWS_PER_CORE], _FP8, tag="at")
            # first row-tile's weights first, then the rest
            nc.sync.dma_start(at_sb[:, :, 0:P], at_dram[:, :, 0:P])
            ft_sb = ft_pool.tile([P, KH, N], _FP8, tag="ft")
            # small first blocks so the first matmuls can start early
            cuts = [0, 1024, 2048, 4096]
            blk = (N - 4096) // ft_split
            cuts += [4096 + j * blk for j in range(1, ft_split)] + [N]
            for j0, j1 in zip(cuts, cuts[1:]):
                nc.sync.dma_start(ft_sb[:, :, j0:j1], ft_dram[:, :, j0:j1])
                if j0 == 0:
                    nc.sync.dma_start(
                        at_sb[:, :, P:ROWS_PER_CORE],
                        at_dram[:, :, P:ROWS_PER_CORE],
                    )

            dummies = [
                d_pool.tile([P, psw], _BF16, name=f"dummy{i}", tag="dummy")
                for i in range(4)
            ]


            for r in range(ROW_TILES):
                pattern = patterns[r % len(patterns)]
                accs = [
                    acc_pool.tile([P, psw], _BF16, name=f"acc{i}", tag="accv")
                    for i in range(2)
                ]
                scores = sc_pool.tile([P, max(n_lse, 1)], _F32, tag="sc")
                seen = [False, False]
                vi = 0
                li = 0
                acc_g = None
                gi = 0
                if pattern.count("G"):
                    acc_g = [
                        acc_pool.tile([P, psw], _BF16, name=f"accg{i}", tag="accg")
                        for i in range(2)
                    ]
                for t in range(n_ps):
                    ps = psum_pool.tile([P, psw], _F32, tag="ps")
                    for h in range(chunks_per_ps):
                        c = (t * psw) // CHUNK + h
                        nc.tensor.matmul(
                            ps[:, h * CHUNK : (h + 1) * CHUNK],
                            at_sb[:, :, r * P : (r + 1) * P],
                            ft_sb[:, :, c * CHUNK : (c + 1) * CHUNK],
                            start=True,
                            stop=True,
                            perf_mode=mybir.MatmulPerfMode.DoubleRow,
                        )
                    ch = pattern[t]
                    if ch == "V":
                        idx = vi % 2
                        a = accs[idx]
                        vi += 1
                        if not seen[idx]:
                            nc.vector.tensor_scalar_max(a[:], ps[:], NEG)
                            seen[idx] = True
                        else:
                            nc.vector.tensor_tensor(a[:], ps[:], a[:], _MAX)
                    elif ch in "AB":
                        s = s_pool.tile([P, psw], _BF16, tag="sb")
                        nc.scalar.copy(s[:], ps[:])
                        idx = vi % 2
                        a = accs[idx]
                        vi += 1
                        if not seen[idx]:
                            raise AssertionError("fold before acc init")
                        nc.vector.tensor_tensor(a[:], s[:], a[:], _MAX)
                    elif ch == "L":
                        nc.scalar.activation(
                            dummies[li % 2][:],
                            ps[:],
                            mybir.ActivationFunctionType.Exp,
                            scale=LSE_SCALE,
                            accum_out=scores[:, li : li + 1],
                        )
                        li += 1
                    elif ch == "G":
                        a_g = acc_g[gi % 2]
                        if gi < 2:
                            # first two G copies initialize the G-accs
                            nc.scalar.activation(
                                a_g[:],
                                ps[:],
                                mybir.ActivationFunctionType.Exp,
                                scale=LSE_SCALE,
                            )
                        else:
                            sg = s_pool.tile([P, psw], _BF16, tag="sg")
                            nc.scalar.activation(
                                sg[:],
                                ps[:],
                                mybir.ActivationFunctionType.Exp,
                                scale=LSE_SCALE,
                            )
                            nc.gpsimd.tensor_tensor(
                                a_g[:], sg[:], a_g[:], _ADD
                            )
                        gi += 1
                assert seen[0] and seen[1] and w_ship == psw
                nc.sync.dma_start(cls_dram[r, :, 0, :], accs[0][:])
                nc.sync.dma_start(cls_dram[r, :, 1, :], accs[1][:])
                if acc_g is not None:
                    nc.sync.dma_start(gcl_dram[r, :, 0, :], acc_g[0][:])
                    if gi > 1:
                        nc.sync.dma_start(gcl_dram[r, :, 1, :], acc_g[1][:])
                if n_lse:
                    nc.sync.dma_start(lse_dram[r], scores[:])

    nc.compile()
    return nc


_NC_CACHE = {}


def _get_nc():
    if "nc" not in _NC_CACHE:
        _NC_CACHE["nc"] = build_nc()
    return _NC_CACHE["nc"]


def make_inputs(feature: np.ndarray):
    """Host-side shard prep: F^T in [P, KH, cols] layout, fp8-quantized."""
    ft = np.ascontiguousarray(
        feature.T.reshape(KH, P, N).transpose(1, 0, 2)
    ).astype(_FP8_NP)
    in_maps = []
    for c in range(N_CORES):
        at = np.ascontiguousarray(
            ft[:, :, c * ROWS_PER_CORE : (c + 1) * ROWS_PER_CORE]
        )
        in_maps.append({"ft": ft, "at": at})
    return in_maps


def run_device(feature: np.ndarray, trace: bool = False):
    """Run the SPMD kernel.

    Returns (cls [N, W] f32 class maxima over non-L columns,
             lse [N, n_lse] f32 chunk scores, results)."""
    nc = _get_nc()
    in_maps = make_inputs(feature)
    res = run_bass_kernel_spmd(
        nc, in_maps, core_ids=list(range(N_CORES)), trace=trace
    )
    cls_parts, lse_parts, g_parts = [], [], []
    for r in res.results:
        c = np.asarray(r["cls"]).astype(np.float32)  # [RT, P, 2, W]
        cls_parts.append(c.max(axis=2).reshape(ROWS_PER_CORE, -1))
        lse_parts.append(np.asarray(r["lse"]).reshape(ROWS_PER_CORE, -1))
        g = np.asarray(r["gcl"]).astype(np.float32)  # [RT, P, 2, W]
        g_parts.append(g.sum(axis=2).reshape(ROWS_PER_CORE, -1))
    return (
        np.concatenate(cls_parts),
        np.concatenate(lse_parts),
        np.concatenate(g_parts),
        res,
    )


def recover_loss(
    feature: np.ndarray,
    cls: np.ndarray,
    lse: np.ndarray,
    gcl: np.ndarray | None = None,
) -> np.float32:
    """Exact argmax recovery + reference loss formula on host.

    cls[g] = per-class maxima (class = col mod W) over columns drained
    through the fold channels; lse[g] = per-L-chunk exp-sum scores for
    columns covered by LSE tiles. Candidates per row: top-K classes,
    the diagonal class, top-C LSE chunks, and the diagonal's own chunk
    if it lies in an LSE tile. All candidates evaluated in exact fp32.
    """
    n, w = feature.shape[0], cls.shape[1]
    B = n // w
    psw = PSW
    feat = np.ascontiguousarray(feature, dtype=np.float32)
    rows = np.arange(n)
    k = min(TOPK, w - 1)
    t_cls = np.argpartition(-cls, k, axis=1)[:, :k].astype(np.int64)

    best_val = np.full(n, -np.inf, dtype=np.float32)
    best_col = np.zeros(n, dtype=np.int64)

    def consider_cols(row_idx: np.ndarray, cols: np.ndarray):
        cd = feat[row_idx] @ feat[cols].T
        self_pos = np.searchsorted(cols, row_idx)
        kk = np.arange(len(row_idx))
        has_self = (self_pos < len(cols)) & (
            cols[np.minimum(self_pos, len(cols) - 1)] == row_idx
        )
        cd[kk[has_self], self_pos[has_self]] = -np.inf
        b = np.argmax(cd, axis=1)
        v = cd[kk, b]
        c = cols[b]
        upd = (v > best_val[row_idx]) | (
            (v == best_val[row_idx]) & (c < best_col[row_idx])
        )
        ri = row_idx[upd]
        best_val[ri] = v[upd]
        best_col[ri] = c[upd]

    # class candidates: top-K device classes (+ exp-domain G classes)
    # + diagonal class
    parts = [t_cls]
    if gcl is not None and gcl.shape[1] == w:
        kg = min(3, w - 1)
        parts.append(
            np.argpartition(-np.nan_to_num(gcl, nan=-np.inf), kg, axis=1)[
                :, :kg
            ].astype(np.int64)
        )
    parts.append((rows % w)[:, None])
    all_cls = np.concatenate(parts, axis=1)
    for j in range(all_cls.shape[1]):
        col = all_cls[:, j]
        order = np.argsort(col, kind="stable")
        bounds = np.searchsorted(col[order], np.arange(w + 1))
        for t in range(w):
            grp = order[bounds[t] : bounds[t + 1]]
            if len(grp):
                consider_cols(grp, t + w * np.arange(B))

    # LSE chunk candidates
    n_ps = N // psw
    lpos = {}  # row-tile index -> list of L tile positions
    for r in range(ROW_TILES):
        pat = PATTERNS[r % len(PATTERNS)]
        lpos[r] = [t for t in range(n_ps) if pat[t] == "L"]
    n_lse = lse.shape[1]
    if n_lse:
        rt = (rows % ROWS_PER_CORE) // P  # row-tile index per row
        lchunks = np.full((n, n_lse), -1, dtype=np.int64)
        for r in range(ROW_TILES):
            sel = rt == r
            for j, t in enumerate(lpos[r]):
                lchunks[sel, j] = t
        # slots without an L tile carry garbage scores; mask them out
        lse = np.where(
            lchunks >= 0, np.nan_to_num(lse, nan=-np.inf), -np.inf
        )
        cc = min(TOPC, n_lse)
        top = np.argpartition(-lse, cc - 1, axis=1)[:, :cc]
        want = np.zeros((n, n_ps), dtype=bool)
        kk = np.arange(n)[:, None]
        want[kk, lchunks[kk, top]] = True
        # diagonal chunk if the row's own column lies in an L tile
        diag_t = rows // psw % n_ps
        in_l = lchunks == diag_t[:, None]
        want[in_l.any(axis=1), diag_t[in_l.any(axis=1)]] = True
        for t in range(n_ps):
            grp = rows[want[:, t]]
            if len(grp):
                consider_cols(grp, np.arange(t * psw, (t + 1) * psw))

    I = best_col
    diff = feat - feat[I] + EPS
    dist = np.sqrt((diff * diff).sum(axis=1))
    loss = -np.mean(np.log(n * dist))
    return np.float32(loss)


def kernel(feature: np.ndarray) -> np.ndarray:
    feature = np.asarray(feature, dtype=np.float32)
    for attempt in range(3):
        try:
            cls, lse, gcl, _res = run_device(feature)
            break
        except Exception:
            # transient device/tunnel hiccups; rebuild and retry
            _NC_CACHE.clear()
            if attempt == 2:
                raise
    return np.asarray(recover_loss(feature, cls, lse, gcl), dtype=np.float32)


if __name__ == "__main__":
    rng = np.random.default_rng(0)
    feature = rng.standard_normal((N, D), dtype=np.float32)
    print("loss:", kernel(feature))


# revision 11
# speedup vs baseline: 1.0512x; 1.0006x over previous
"""Trainium2 kernel v3 for nn_Loss_26886495273741 (retrieval_knn).

reference:
    dots = feature @ feature.T          # [n, n], n=16384, d=256
    dots[diag] = -1
    I = argmax(dots, axis=1)
    loss = -mean(log(n * ||feature - feature[I] + 1e-6||_2))

Device strategy (8 NeuronCores, SPMD, host-replicated "all-gather"):
  * Rows sharded: core c owns rows [c*2048, (c+1)*2048).
  * fp8 DoubleRow matmuls fill PSUM tiles [128, 1024] fp32 (x4 bufs)
    with the row-block of dots, 16 tiles per 128-row tile.
  * Hardware constraints (verified): only ACT and DVE can read PSUM and
    only one PSUM operand per instruction, so the drain is split:
      V: DVE absorb  tensor_tensor max(ps, acc_v) -> acc_v   (bf16 acc)
      A: ACT copy -> bf16 s;  Pool folds s into acc_p
      B: ACT copy -> bf16 s;  DVE folds s into acc_v
      L: ACT exp(s*dot) with fp32 accumulator -> per-chunk LSE score
         (self-contained; those columns are covered by chunk scores,
         not by the class fold)
  * acc_v/acc_p are halved to W classes (DVE / Pool) and shipped; LSE
    chunk scores ship as fp32. Host: top-K classes + top LSE chunks +
    diagonal class/chunk, exact fp32 candidate eval, reference loss.
"""

import os
import sys

import numpy as np

_jp = os.environ.get("JAX_PLATFORMS")
if _jp is not None and "axon" not in _jp:
    os.environ["JAX_PLATFORMS"] = "axon," + _jp

try:
    import concourse.bass as bass  # noqa: F401
except ImportError:  # grading env runs from a bare directory
    sys.path.insert(0, "/opt/trn_rl_repo")

import ml_dtypes  # noqa: F401

import concourse.bass as bass
import concourse.mybir as mybir
import concourse.tile as tile
from concourse import bacc
from concourse.bass_utils import run_bass_kernel_spmd

# Problem geometry (hardcoded per spec.json: feature [16384, 256] f32).
N = 16384
D = 256
N_CORES = 8
ROWS_PER_CORE = N // N_CORES  # 2048
P = 128
ROW_TILES = ROWS_PER_CORE // P  # 16
CHUNK = 512  # matmul free dim == one PSUM bank (fp32)
KH = D // P  # 2 contraction halves packed for DoubleRow

EPS = 1e-6

_BF16 = mybir.dt.bfloat16
_F32 = mybir.dt.float32
_FP8 = mybir.dt.float8e4
_FP8_NP = mybir.dt.np(_FP8)

_MAX = mybir.AluOpType.max
_ADD = mybir.AluOpType.add

# --- tunables (swept offline with TimelineSim; best kept hardcoded) ---
PSW = 1024  # psum tile width (2 banks, 4 bufs)
# per-row-tile channel patterns, cycled over row tiles.
# V=DVE absorb, A=ACT copy+Pool fold, B=ACT copy+DVE fold, L=ACT LSE
PATTERNS = (
    ("VLVGVLVLVLVLVLVL",)
    + ("VGVGVGVGVGVGVGVG", "VGVGVGVGVGVGVGGG") * 7
    + ("VGVLVLVGVLVLVLVL",)
)
W_SHIP = 1024  # classes shipped per row (= PSW: no halving)
FT_SPLIT = 12  # column blocks for the big ft load
TOPK = 4  # host-side top-k classes
TOPC = 3  # host-side top-k LSE chunks
LSE_SCALE = 0.275  # exp scale for LSE scores

NEG = -3.0e38


def build_nc(
    psw: int = PSW,
    patterns: tuple = PATTERNS,
    w_ship: int = W_SHIP,
    ft_split: int = FT_SPLIT,
    psum_bufs: int = 0,
):
    n_ps = N // psw  # psum tiles per row-tile
    chunks_per_ps = psw // CHUNK
    if not psum_bufs:
        psum_bufs = (16 * 1024) // (psw * 4)  # fill all 8 banks
    n_lse = max(pat.count("L") for pat in patterns)
    n_g = max(pat.count("G") for pat in patterns)
    for pat in patterns:
        assert len(pat) == n_ps, pat
        assert all(c in "VABLG" for c in pat)
        assert pat[0] == "V", "first V initializes acc_v"

    nc = bacc.Bacc("TRN2", target_bir_lowering=False, debug=False)

    ft_dram = nc.dram_tensor("ft", [P, KH, N], _FP8, kind="ExternalInput")
    at_dram = nc.dram_tensor(
        "at", [P, KH, ROWS_PER_CORE], _FP8, kind="ExternalInput"
    )
    cls_dram = nc.dram_tensor(
        "cls", [ROW_TILES, P, 2, w_ship], _BF16, kind="ExternalOutput"
    )
    lse_dram = nc.dram_tensor(
        "lse", [ROW_TILES, P, max(n_lse, 1)], _F32, kind="ExternalOutput"
    )
    gcl_dram = nc.dram_tensor(
        "gcl", [ROW_TILES, P, 5, w_ship if n_g else 1], _BF16,
        kind="ExternalOutput",
    )

    with tile.TileContext(nc) as tc:
        with (
            tc.tile_pool(name="ft_pool", bufs=1) as ft_pool,
            tc.tile_pool(name="at_pool", bufs=1) as at_pool,
            tc.tile_pool(name="s_pool", bufs=8) as s_pool,
            tc.tile_pool(name="d_pool", bufs=4) as d_pool,
            tc.tile_pool(name="acc_pool", bufs=14) as acc_pool,
            tc.tile_pool(name="sc_pool", bufs=2) as sc_pool,
            tc.tile_pool(name="psum", bufs=psum_bufs, space="PSUM") as psum_pool,
        ):
            at_sb = at_pool.tile([P, KH, ROWS_PER_CORE], _FP8, tag="at")
            # first row-tile's weights first, then the rest
            nc.sync.dma_start(at_sb[:, :, 0:P], at_dram[:, :, 0:P])
            ft_sb = ft_pool.tile([P, KH, N], _FP8, tag="ft")
            # small first blocks so the first matmuls can start early
            cuts = [0, 1024, 2048, 4096]
            blk = (N - 4096) // ft_split
            cuts += [4096 + j * blk for j in range(1, ft_split)] + [N]
            for j0, j1 in zip(cuts, cuts[1:]):
                nc.sync.dma_start(ft_sb[:, :, j0:j1], ft_dram[:, :, j0:j1])
                if j0 == 0:
                    nc.sync.dma_start(
                        at_sb[:, :, P:ROWS_PER_CORE],
                        at_dram[:, :, P:ROWS_PER_CORE],
                    )

            dummies = [
                d_pool.tile([P, psw], _BF16, name=f"dummy{i}", tag="dummy")
                for i in range(4)
            ]


            for r in range(ROW_TILES):
                pattern = patterns[r % len(patterns)]
                accs = [
                    acc_pool.tile([P, psw], _BF16, name=f"acc{i}", tag="accv")
                    for i in range(2)
                ]
                scores = None
                if "L" in pattern:
                    scores = sc_pool.tile(
                        [P, max(n_lse, 1)], _F32, tag="sc"
                    )
                seen = [False, False]
                vi = 0
                li = 0
                acc_g = None
                gi = 0
                n_ag = min(pattern.count("G"), 5)
                if n_ag:
                    acc_g = [
                        acc_pool.tile([P, psw], _BF16, name=f"accg{i}", tag="accg")
                        for i in range(n_ag)
                    ]
                for t in range(n_ps):
                    ps = psum_pool.tile([P, psw], _F32, tag="ps")
                    for h in range(chunks_per_ps):
                        c = (t * psw) // CHUNK + h
                        nc.tensor.matmul(
                            ps[:, h * CHUNK : (h + 1) * CHUNK],
                            at_sb[:, :, r * P : (r + 1) * P],
                            ft_sb[:, :, c * CHUNK : (c + 1) * CHUNK],
                            start=True,
                            stop=True,
                            perf_mode=mybir.MatmulPerfMode.DoubleRow,
                        )
                    ch = pattern[t]
                    if ch == "V":
                        idx = vi % 2
                        a = accs[idx]
                        vi += 1
                        if not seen[idx]:
                            nc.vector.tensor_scalar_max(a[:], ps[:], NEG)
                            seen[idx] = True
                        else:
                            nc.vector.tensor_tensor(a[:], ps[:], a[:], _MAX)
                    elif ch in "AB":
                        s = s_pool.tile([P, psw], _BF16, tag="sb")
                        nc.scalar.copy(s[:], ps[:])
                        idx = vi % 2
                        a = accs[idx]
                        vi += 1
                        if not seen[idx]:
                            raise AssertionError("fold before acc init")
                        nc.vector.tensor_tensor(a[:], s[:], a[:], _MAX)
                    elif ch == "L":
                        nc.scalar.activation(
                            dummies[li % 2][:],
                            ps[:],
                            mybir.ActivationFunctionType.Exp,
                            scale=LSE_SCALE,
                            accum_out=scores[:, li : li + 1],
                        )
                        li += 1
                    elif ch == "G":
                        a_g = acc_g[gi % n_ag]
                        if gi < n_ag:
                            # first two G copies initialize the G-accs
                            nc.scalar.activation(
                                a_g[:],
                                ps[:],
                                mybir.ActivationFunctionType.Exp,
                                scale=LSE_SCALE,
                            )
                        else:
                            sg = s_pool.tile([P, psw], _BF16, tag="sg")
                            nc.scalar.activation(
                                sg[:],
                                ps[:],
                                mybir.ActivationFunctionType.Exp,
                                scale=LSE_SCALE,
                            )
                            nc.gpsimd.tensor_tensor(
                                a_g[:], sg[:], a_g[:], _ADD
                            )
                        gi += 1
                assert seen[0] and seen[1] and w_ship == psw
                nc.sync.dma_start(cls_dram[r, :, 0, :], accs[0][:])
                nc.sync.dma_start(cls_dram[r, :, 1, :], accs[1][:])
                if acc_g is not None:
                    for i in range(min(gi, n_ag)):
                        nc.sync.dma_start(gcl_dram[r, :, i, :], acc_g[i][:])
                if n_lse and scores is not None:
                    nc.sync.dma_start(lse_dram[r], scores[:])

    nc.compile()
    return nc


_NC_CACHE = {}


def _get_nc():
    if "nc" not in _NC_CACHE:
        _NC_CACHE["nc"] = build_nc()
    return _NC_CACHE["nc"]


def make_inputs(feature: np.ndarray):
    """Host-side shard prep: F^T in [P, KH, cols] layout, fp8-quantized."""
    ft = np.ascontiguousarray(
        feature.T.reshape(KH, P, N).transpose(1, 0, 2)
    ).astype(_FP8_NP)
    in_maps = []
    for c in range(N_CORES):
        at = np.ascontiguousarray(
            ft[:, :, c * ROWS_PER_CORE : (c + 1) * ROWS_PER_CORE]
        )
        in_maps.append({"ft": ft, "at": at})
    return in_maps


def run_device(feature: np.ndarray, trace: bool = False):
    """Run the SPMD kernel.

    Returns (cls [N, W] f32 class maxima over non-L columns,
             lse [N, n_lse] f32 chunk scores, results)."""
    nc = _get_nc()
    in_maps = make_inputs(feature)
    res = run_bass_kernel_spmd(
        nc, in_maps, core_ids=list(range(N_CORES)), trace=trace
    )
    cls_parts, lse_parts, g_parts = [], [], []
    for r in res.results:
        c = np.asarray(r["cls"]).astype(np.float32)  # [RT, P, 2, W]
        cls_parts.append(c.max(axis=2).reshape(ROWS_PER_CORE, -1))
        lse_parts.append(np.asarray(r["lse"]).reshape(ROWS_PER_CORE, -1))
        g = np.asarray(r["gcl"]).astype(np.float32)  # [RT, P, 5, W]
        g_parts.append(g.sum(axis=2).reshape(ROWS_PER_CORE, -1))
    return (
        np.concatenate(cls_parts),
        np.concatenate(lse_parts),
        np.concatenate(g_parts),
        res,
    )


def recover_loss(
    feature: np.ndarray,
    cls: np.ndarray,
    lse: np.ndarray,
    gcl: np.ndarray | None = None,
) -> np.float32:
    """Exact argmax recovery + reference loss formula on host.

    cls[g] = per-class maxima (class = col mod W) over columns drained
    through the fold channels; lse[g] = per-L-chunk exp-sum scores for
    columns covered by LSE tiles. Candidates per row: top-K classes,
    the diagonal class, top-C LSE chunks, and the diagonal's own chunk
    if it lies in an LSE tile. All candidates evaluated in exact fp32.
    """
    n, w = feature.shape[0], cls.shape[1]
    B = n // w
    psw = PSW
    feat = np.ascontiguousarray(feature, dtype=np.float32)
    rows = np.arange(n)
    k = min(TOPK, w - 1)
    t_cls = np.argpartition(-cls, k, axis=1)[:, :k].astype(np.int64)

    best_val = np.full(n, -np.inf, dtype=np.float32)
    best_col = np.zeros(n, dtype=np.int64)

    def consider_cols(row_idx: np.ndarray, cols: np.ndarray):
        cd = feat[row_idx] @ feat[cols].T
        self_pos = np.searchsorted(cols, row_idx)
        kk = np.arange(len(row_idx))
        has_self = (self_pos < len(cols)) & (
            cols[np.minimum(self_pos, len(cols) - 1)] == row_idx
        )
        cd[kk[has_self], self_pos[has_self]] = -np.inf
        b = np.argmax(cd, axis=1)
        v = cd[kk, b]
        c = cols[b]
        upd = (v > best_val[row_idx]) | (
            (v == best_val[row_idx]) & (c < best_col[row_idx])
        )
        ri = row_idx[upd]
        best_val[ri] = v[upd]
        best_col[ri] = c[upd]

    # class candidates: top-K device classes (+ exp-domain G classes)
    # + diagonal class
    parts = [t_cls]
    if gcl is not None and gcl.shape[1] == w:
        kg = min(3, w - 1)
        parts.append(
            np.argpartition(-np.nan_to_num(gcl, nan=-np.inf), kg, axis=1)[
                :, :kg
            ].astype(np.int64)
        )
    parts.append((rows % w)[:, None])
    all_cls = np.concatenate(parts, axis=1)
    for j in range(all_cls.shape[1]):
        col = all_cls[:, j]
        order = np.argsort(col, kind="stable")
        bounds = np.searchsorted(col[order], np.arange(w + 1))
        for t in range(w):
            grp = order[bounds[t] : bounds[t + 1]]
            if len(grp):
                consider_cols(grp, t + w * np.arange(B))

    # LSE chunk candidates
    n_ps = N // psw
    lpos = {}  # row-tile index -> list of L tile positions
    for r in range(ROW_TILES):
        pat = PATTERNS[r % len(PATTERNS)]
        lpos[r] = [t for t in range(n_ps) if pat[t] == "L"]
    n_lse = lse.shape[1] if any("L" in p for p in PATTERNS) else 0
    if n_lse:
        rt = (rows % ROWS_PER_CORE) // P  # row-tile index per row
        lchunks = np.full((n, n_lse), -1, dtype=np.int64)
        for r in range(ROW_TILES):
            sel = rt == r
            for j, t in enumerate(lpos[r]):
                lchunks[sel, j] = t
        # slots without an L tile carry garbage scores; mask them out
        lse = np.where(
            lchunks >= 0, np.nan_to_num(lse, nan=-np.inf), -np.inf
        )
        cc = min(TOPC, n_lse)
        top = np.argpartition(-lse, cc - 1, axis=1)[:, :cc]
        want = np.zeros((n, n_ps), dtype=bool)
        kk = np.arange(n)[:, None]
        want[kk, lchunks[kk, top]] = True
        # diagonal chunk if the row's own column lies in an L tile
        diag_t = rows // psw % n_ps
        in_l = lchunks == diag_t[:, None]
        want[in_l.any(axis=1), diag_t[in_l.any(axis=1)]] = True
        for t in range(n_ps):
            grp = rows[want[:, t]]
            if len(grp):
                consider_cols(grp, np.arange(t * psw, (t + 1) * psw))

    I = best_col
    diff = feat - feat[I] + EPS
    dist = np.sqrt((diff * diff).sum(axis=1))
    loss = -np.mean(np.log(n * dist))
    return np.float32(loss)


def kernel(feature: np.ndarray) -> np.ndarray:
    feature = np.asarray(feature, dtype=np.float32)
    for attempt in range(3):
        try:
            cls, lse, gcl, _res = run_device(feature)
            break
        except Exception:
            # transient device/tunnel hiccups; rebuild and retry
            _NC_CACHE.clear()
            if attempt == 2:
                raise
    return np.asarray(recover_loss(feature, cls, lse, gcl), dtype=np.float32)


if __name__ == "__main__":
    rng = np.random.default_rng(0)
    feature = rng.standard_normal((N, D), dtype=np.float32)
    print("loss:", kernel(feature))


# revision 12
# speedup vs baseline: 1.0579x; 1.0063x over previous
"""Trainium2 kernel v3 for nn_Loss_26886495273741 (retrieval_knn).

reference:
    dots = feature @ feature.T          # [n, n], n=16384, d=256
    dots[diag] = -1
    I = argmax(dots, axis=1)
    loss = -mean(log(n * ||feature - feature[I] + 1e-6||_2))

Device strategy (8 NeuronCores, SPMD, host-replicated "all-gather"):
  * Rows sharded: core c owns rows [c*2048, (c+1)*2048).
  * fp8 DoubleRow matmuls fill PSUM tiles [128, 1024] fp32 (x4 bufs)
    with the row-block of dots, 16 tiles per 128-row tile.
  * Hardware constraints (verified): only ACT and DVE can read PSUM and
    only one PSUM operand per instruction, so the drain is split:
      V: DVE absorb  tensor_tensor max(ps, acc_v) -> acc_v   (bf16 acc)
      A: ACT copy -> bf16 s;  Pool folds s into acc_p
      B: ACT copy -> bf16 s;  DVE folds s into acc_v
      L: ACT exp(s*dot) with fp32 accumulator -> per-chunk LSE score
         (self-contained; those columns are covered by chunk scores,
         not by the class fold)
  * acc_v/acc_p are halved to W classes (DVE / Pool) and shipped; LSE
    chunk scores ship as fp32. Host: top-K classes + top LSE chunks +
    diagonal class/chunk, exact fp32 candidate eval, reference loss.
"""

import os
import sys

import numpy as np

_jp = os.environ.get("JAX_PLATFORMS")
if _jp is not None and "axon" not in _jp:
    os.environ["JAX_PLATFORMS"] = "axon," + _jp

try:
    import concourse.bass as bass  # noqa: F401
except ImportError:  # grading env runs from a bare directory
    sys.path.insert(0, "/opt/trn_rl_repo")

import ml_dtypes  # noqa: F401

import concourse.bass as bass
import concourse.mybir as mybir
import concourse.tile as tile
from concourse import bacc
from concourse.bass_utils import run_bass_kernel_spmd

# Problem geometry (hardcoded per spec.json: feature [16384, 256] f32).
N = 16384
D = 256
N_CORES = 8
ROWS_PER_CORE = N // N_CORES  # 2048
P = 128
ROW_TILES = ROWS_PER_CORE // P  # 16
CHUNK = 512  # matmul free dim == one PSUM bank (fp32)
KH = D // P  # 2 contraction halves packed for DoubleRow

EPS = 1e-6

_BF16 = mybir.dt.bfloat16
_F32 = mybir.dt.float32
_FP8 = mybir.dt.float8e4
_FP8_NP = mybir.dt.np(_FP8)

_MAX = mybir.AluOpType.max
_ADD = mybir.AluOpType.add

# --- tunables (swept offline with TimelineSim; best kept hardcoded) ---
PSW = 1024  # psum tile width (2 banks, 4 bufs)
# per-row-tile channel patterns, cycled over row tiles.
# V=DVE absorb, A=ACT copy+Pool fold, B=ACT copy+DVE fold, L=ACT LSE
PATTERNS = (
    ("VGVLVLVGVLVLVLVL",)
    + ("VGVGVGVGVGVGVGVG", "VGVGVGVGVGVGVGGG") * 7
    + ("VGVLVLVGVLVLVLVL",)
)
W_SHIP = 1024  # classes shipped per row (= PSW: no halving)
FT_SPLIT = 12  # column blocks for the big ft load
TOPK = 4  # host-side top-k classes
TOPC = 3  # host-side top-k LSE chunks
LSE_SCALE = 0.275  # exp scale for LSE scores

NEG = -3.0e38


def build_nc(
    psw: int = PSW,
    patterns: tuple = PATTERNS,
    w_ship: int = W_SHIP,
    ft_split: int = FT_SPLIT,
    psum_bufs: int = 0,
):
    n_ps = N // psw  # psum tiles per row-tile
    chunks_per_ps = psw // CHUNK
    if not psum_bufs:
        psum_bufs = (16 * 1024) // (psw * 4)  # fill all 8 banks
    n_lse = max(pat.count("L") for pat in patterns)
    n_g = max(pat.count("G") for pat in patterns)
    for pat in patterns:
        assert len(pat) == n_ps, pat
        assert all(c in "VABLG" for c in pat)
        assert pat[0] == "V", "first V initializes acc_v"

    nc = bacc.Bacc("TRN2", target_bir_lowering=False, debug=False)

    ft_dram = nc.dram_tensor("ft", [P, KH, N], _FP8, kind="ExternalInput")
    at_dram = nc.dram_tensor(
        "at", [P, KH, ROWS_PER_CORE], _FP8, kind="ExternalInput"
    )
    cls_dram = nc.dram_tensor(
        "cls", [ROW_TILES, P, 2, w_ship], _BF16, kind="ExternalOutput"
    )
    lse_dram = nc.dram_tensor(
        "lse", [ROW_TILES, P, max(n_lse, 1)], _F32, kind="ExternalOutput"
    )
    gcl_dram = nc.dram_tensor(
        "gcl", [ROW_TILES, P, 5, w_ship if n_g else 1], _BF16,
        kind="ExternalOutput",
    )

    with tile.TileContext(nc) as tc:
        with (
            tc.tile_pool(name="ft_pool", bufs=1) as ft_pool,
            tc.tile_pool(name="at_pool", bufs=1) as at_pool,
            tc.tile_pool(name="s_pool", bufs=8) as s_pool,
            tc.tile_pool(name="d_pool", bufs=4) as d_pool,
            tc.tile_pool(name="acc_pool", bufs=14) as acc_pool,
            tc.tile_pool(name="sc_pool", bufs=2) as sc_pool,
            tc.tile_pool(name="psum", bufs=psum_bufs, space="PSUM") as psum_pool,
        ):
            at_sb = at_pool.tile([P, KH, ROWS_PER_CORE], _FP8, tag="at")
            # first row-tile's weights first, then the rest
            nc.sync.dma_start(at_sb[:, :, 0:P], at_dram[:, :, 0:P])
            ft_sb = ft_pool.tile([P, KH, N], _FP8, tag="ft")
            # small first blocks so the first matmuls can start early
            cuts = [0, 1024, 2048, 3072, 4096]
            blk = (N - 4096) // ft_split
            cuts += [4096 + j * blk for j in range(1, ft_split)] + [N]
            for j0, j1 in zip(cuts, cuts[1:]):
                nc.sync.dma_start(ft_sb[:, :, j0:j1], ft_dram[:, :, j0:j1])
                if j0 == 3072:
                    nc.sync.dma_start(
                        at_sb[:, :, P:ROWS_PER_CORE],
                        at_dram[:, :, P:ROWS_PER_CORE],
                    )

            dummies = [
                d_pool.tile([P, psw], _BF16, name=f"dummy{i}", tag="dummy")
                for i in range(4)
            ]


            for r in range(ROW_TILES):
                pattern = patterns[r % len(patterns)]
                accs = [
                    acc_pool.tile([P, psw], _BF16, name=f"acc{i}", tag="accv")
                    for i in range(2)
                ]
                scores = None
                if "L" in pattern:
                    scores = sc_pool.tile(
                        [P, max(n_lse, 1)], _F32, tag="sc"
                    )
                seen = [False, False]
                vi = 0
                li = 0
                acc_g = None
                gi = 0
                n_ag = min(pattern.count("G"), 5)
                if n_ag:
                    acc_g = [
                        acc_pool.tile([P, psw], _BF16, name=f"accg{i}", tag="accg")
                        for i in range(n_ag)
                    ]
                for t in range(n_ps):
                    ps = psum_pool.tile([P, psw], _F32, tag="ps")
                    for h in range(chunks_per_ps):
                        c = (t * psw) // CHUNK + h
                        nc.tensor.matmul(
                            ps[:, h * CHUNK : (h + 1) * CHUNK],
                            at_sb[:, :, r * P : (r + 1) * P],
                            ft_sb[:, :, c * CHUNK : (c + 1) * CHUNK],
                            start=True,
                            stop=True,
                            perf_mode=mybir.MatmulPerfMode.DoubleRow,
                        )
                    ch = pattern[t]
                    if ch == "V":
                        idx = vi % 2
                        a = accs[idx]
                        vi += 1
                        if not seen[idx]:
                            nc.vector.tensor_scalar_max(a[:], ps[:], NEG)
                            seen[idx] = True
                        else:
                            nc.vector.tensor_tensor(a[:], ps[:], a[:], _MAX)
                    elif ch in "AB":
                        s = s_pool.tile([P, psw], _BF16, tag="sb")
                        nc.scalar.copy(s[:], ps[:])
                        idx = vi % 2
                        a = accs[idx]
                        vi += 1
                        if not seen[idx]:
                            raise AssertionError("fold before acc init")
                        nc.vector.tensor_tensor(a[:], s[:], a[:], _MAX)
                    elif ch == "L":
                        nc.scalar.activation(
                            dummies[li % 2][:],
                            ps[:],
                            mybir.ActivationFunctionType.Exp,
                            scale=LSE_SCALE,
                            accum_out=scores[:, li : li + 1],
                        )
                        li += 1
                    elif ch == "G":
                        a_g = acc_g[gi % n_ag]
                        if gi < n_ag:
                            # first two G copies initialize the G-accs
                            nc.scalar.activation(
                                a_g[:],
                                ps[:],
                                mybir.ActivationFunctionType.Exp,
                                scale=LSE_SCALE,
                            )
                        else:
                            sg = s_pool.tile([P, psw], _BF16, tag="sg")
                            nc.scalar.activation(
                                sg[:],
                                ps[:],
                                mybir.ActivationFunctionType.Exp,
                                scale=LSE_SCALE,
                            )
                            nc.gpsimd.tensor_tensor(
                                a_g[:], sg[:], a_g[:], _ADD
                            )
                        gi += 1
                assert seen[0] and seen[1] and w_ship == psw
                nc.sync.dma_start(cls_dram[r, :, 0, :], accs[0][:])
                nc.sync.dma_start(cls_dram[r, :, 1, :], accs[1][:])
                if acc_g is not None:
                    for i in range(min(gi, n_ag)):
                        nc.sync.dma_start(gcl_dram[r, :, i, :], acc_g[i][:])
                if n_lse and scores is not None:
                    nc.sync.dma_start(lse_dram[r], scores[:])

    nc.compile()
    return nc


_NC_CACHE = {}


def _get_nc():
    if "nc" not in _NC_CACHE:
        _NC_CACHE["nc"] = build_nc()
    return _NC_CACHE["nc"]


def make_inputs(feature: np.ndarray):
    """Host-side shard prep: F^T in [P, KH, cols] layout, fp8-quantized."""
    ft = np.ascontiguousarray(
        feature.T.reshape(KH, P, N).transpose(1, 0, 2)
    ).astype(_FP8_NP)
    in_maps = []
    for c in range(N_CORES):
        at = np.ascontiguousarray(
            ft[:, :, c * ROWS_PER_CORE : (c + 1) * ROWS_PER_CORE]
        )
        in_maps.append({"ft": ft, "at": at})
    return in_maps


def run_device(feature: np.ndarray, trace: bool = False):
    """Run the SPMD kernel.

    Returns (cls [N, W] f32 class maxima over non-L columns,
             lse [N, n_lse] f32 chunk scores, results)."""
    nc = _get_nc()
    in_maps = make_inputs(feature)
    res = run_bass_kernel_spmd(
        nc, in_maps, core_ids=list(range(N_CORES)), trace=trace
    )
    cls_parts, lse_parts, g_parts = [], [], []
    for r in res.results:
        c = np.asarray(r["cls"]).astype(np.float32)  # [RT, P, 2, W]
        cls_parts.append(c.max(axis=2).reshape(ROWS_PER_CORE, -1))
        lse_parts.append(np.asarray(r["lse"]).reshape(ROWS_PER_CORE, -1))
        g = np.asarray(r["gcl"]).astype(np.float32)  # [RT, P, 5, W]
        g_parts.append(g.sum(axis=2).reshape(ROWS_PER_CORE, -1))
    return (
        np.concatenate(cls_parts),
        np.concatenate(lse_parts),
        np.concatenate(g_parts),
        res,
    )


def recover_loss(
    feature: np.ndarray,
    cls: np.ndarray,
    lse: np.ndarray,
    gcl: np.ndarray | None = None,
) -> np.float32:
    """Exact argmax recovery + reference loss formula on host.

    cls[g] = per-class maxima (class = col mod W) over columns drained
    through the fold channels; lse[g] = per-L-chunk exp-sum scores for
    columns covered by LSE tiles. Candidates per row: top-K classes,
    the diagonal class, top-C LSE chunks, and the diagonal's own chunk
    if it lies in an LSE tile. All candidates evaluated in exact fp32.
    """
    n, w = feature.shape[0], cls.shape[1]
    B = n // w
    psw = PSW
    feat = np.ascontiguousarray(feature, dtype=np.float32)
    rows = np.arange(n)
    k = min(TOPK, w - 1)
    t_cls = np.argpartition(-cls, k, axis=1)[:, :k].astype(np.int64)

    best_val = np.full(n, -np.inf, dtype=np.float32)
    best_col = np.zeros(n, dtype=np.int64)

    def consider_cols(row_idx: np.ndarray, cols: np.ndarray):
        cd = feat[row_idx] @ feat[cols].T
        self_pos = np.searchsorted(cols, row_idx)
        kk = np.arange(len(row_idx))
        has_self = (self_pos < len(cols)) & (
            cols[np.minimum(self_pos, len(cols) - 1)] == row_idx
        )
        cd[kk[has_self], self_pos[has_self]] = -np.inf
        b = np.argmax(cd, axis=1)
        v = cd[kk, b]
        c = cols[b]
        upd = (v > best_val[row_idx]) | (
            (v == best_val[row_idx]) & (c < best_col[row_idx])
        )
        ri = row_idx[upd]
        best_val[ri] = v[upd]
        best_col[ri] = c[upd]

    # class candidates: top-K device classes (+ exp-domain G classes)
    # + diagonal class
    parts = [t_cls]
    if gcl is not None and gcl.shape[1] == w:
        kg = min(3, w - 1)
        parts.append(
            np.argpartition(-np.nan_to_num(gcl, nan=-np.inf), kg, axis=1)[
                :, :kg
            ].astype(np.int64)
        )
    parts.append((rows % w)[:, None])
    all_cls = np.concatenate(parts, axis=1)
    for j in range(all_cls.shape[1]):
        col = all_cls[:, j]
        order = np.argsort(col, kind="stable")
        bounds = np.searchsorted(col[order], np.arange(w + 1))
        for t in range(w):
            grp = order[bounds[t] : bounds[t + 1]]
            if len(grp):
                consider_cols(grp, t + w * np.arange(B))

    # LSE chunk candidates
    n_ps = N // psw
    lpos = {}  # row-tile index -> list of L tile positions
    for r in range(ROW_TILES):
        pat = PATTERNS[r % len(PATTERNS)]
        lpos[r] = [t for t in range(n_ps) if pat[t] == "L"]
    n_lse = lse.shape[1] if any("L" in p for p in PATTERNS) else 0
    if n_lse:
        rt = (rows % ROWS_PER_CORE) // P  # row-tile index per row
        lchunks = np.full((n, n_lse), -1, dtype=np.int64)
        for r in range(ROW_TILES):
            sel = rt == r
            for j, t in enumerate(lpos[r]):
                lchunks[sel, j] = t
        # slots without an L tile carry garbage scores; mask them out
        lse = np.where(
            lchunks >= 0, np.nan_to_num(lse, nan=-np.inf), -np.inf
        )
        cc = min(TOPC, n_lse)
        top = np.argpartition(-lse, cc - 1, axis=1)[:, :cc]
        want = np.zeros((n, n_ps), dtype=bool)
        kk = np.arange(n)[:, None]
        want[kk, lchunks[kk, top]] = True
        # diagonal chunk if the row's own column lies in an L tile
        diag_t = rows // psw % n_ps
        in_l = lchunks == diag_t[:, None]
        want[in_l.any(axis=1), diag_t[in_l.any(axis=1)]] = True
        for t in range(n_ps):
            grp = rows[want[:, t]]
            if len(grp):
                consider_cols(grp, np.arange(t * psw, (t + 1) * psw))

    I = best_col
    diff = feat - feat[I] + EPS
    dist = np.sqrt((diff * diff).sum(axis=1))
    loss = -np.mean(np.log(n * dist))
    return np.float32(loss)


def kernel(feature: np.ndarray) -> np.ndarray:
    feature = np.asarray(feature, dtype=np.float32)
    for attempt in range(3):
        try:
            cls, lse, gcl, _res = run_device(feature)
            break
        except Exception:
            # transient device/tunnel hiccups; rebuild and retry
            _NC_CACHE.clear()
            if attempt == 2:
                raise
    return np.asarray(recover_loss(feature, cls, lse, gcl), dtype=np.float32)


if __name__ == "__main__":
    rng = np.random.default_rng(0)
    feature = rng.standard_normal((N, D), dtype=np.float32)
    print("loss:", kernel(feature))


# revision 13
# speedup vs baseline: 1.0586x; 1.0006x over previous
"""Trainium2 kernel v3 for nn_Loss_26886495273741 (retrieval_knn).

reference:
    dots = feature @ feature.T          # [n, n], n=16384, d=256
    dots[diag] = -1
    I = argmax(dots, axis=1)
    loss = -mean(log(n * ||feature - feature[I] + 1e-6||_2))

Device strategy (8 NeuronCores, SPMD, host-replicated "all-gather"):
  * Rows sharded: core c owns rows [c*2048, (c+1)*2048).
  * fp8 DoubleRow matmuls fill PSUM tiles [128, 1024] fp32 (x4 bufs)
    with the row-block of dots, 16 tiles per 128-row tile.
  * Hardware constraints (verified): only ACT and DVE can read PSUM and
    only one PSUM operand per instruction, so the drain is split:
      V: DVE absorb  tensor_tensor max(ps, acc_v) -> acc_v   (bf16 acc)
      A: ACT copy -> bf16 s;  Pool folds s into acc_p
      B: ACT copy -> bf16 s;  DVE folds s into acc_v
      L: ACT exp(s*dot) with fp32 accumulator -> per-chunk LSE score
         (self-contained; those columns are covered by chunk scores,
         not by the class fold)
  * acc_v/acc_p are halved to W classes (DVE / Pool) and shipped; LSE
    chunk scores ship as fp32. Host: top-K classes + top LSE chunks +
    diagonal class/chunk, exact fp32 candidate eval, reference loss.
"""

import os
import sys

import numpy as np

_jp = os.environ.get("JAX_PLATFORMS")
if _jp is not None and "axon" not in _jp:
    os.environ["JAX_PLATFORMS"] = "axon," + _jp

try:
    import concourse.bass as bass  # noqa: F401
except ImportError:  # grading env runs from a bare directory
    sys.path.insert(0, "/opt/trn_rl_repo")

import ml_dtypes  # noqa: F401

import concourse.bass as bass
import concourse.mybir as mybir
import concourse.tile as tile
from concourse import bacc
from concourse.bass_utils import run_bass_kernel_spmd

# Problem geometry (hardcoded per spec.json: feature [16384, 256] f32).
N = 16384
D = 256
N_CORES = 8
ROWS_PER_CORE = N // N_CORES  # 2048
P = 128
ROW_TILES = ROWS_PER_CORE // P  # 16
CHUNK = 512  # matmul free dim == one PSUM bank (fp32)
KH = D // P  # 2 contraction halves packed for DoubleRow

EPS = 1e-6

_BF16 = mybir.dt.bfloat16
_F32 = mybir.dt.float32
_FP8 = mybir.dt.float8e4
_FP8_NP = mybir.dt.np(_FP8)

_MAX = mybir.AluOpType.max
_ADD = mybir.AluOpType.add

# --- tunables (swept offline with TimelineSim; best kept hardcoded) ---
PSW = 1024  # psum tile width (2 banks, 4 bufs)
# per-row-tile channel patterns, cycled over row tiles.
# V=DVE absorb, A=ACT copy+Pool fold, B=ACT copy+DVE fold, L=ACT LSE
PATTERNS = (
    ("VGVLVLVGVLVLVLVL",)
    + ("VGVGVGVGVGVGVGVG", "VGVGVGVGVGVGVGGG") * 7
    + ("VGVLVLVGVLVLVLVL",)
)
W_SHIP = 1024  # classes shipped per row (= PSW: no halving)
FT_SPLIT = 12  # column blocks for the big ft load
TOPK = 4  # host-side top-k classes
TOPC = 3  # host-side top-k LSE chunks
LSE_SCALE = 0.275  # exp scale for LSE scores

NEG = -3.0e38


def build_nc(
    psw: int = PSW,
    patterns: tuple = PATTERNS,
    w_ship: int = W_SHIP,
    ft_split: int = FT_SPLIT,
    psum_bufs: int = 0,
):
    n_ps = N // psw  # psum tiles per row-tile
    chunks_per_ps = psw // CHUNK
    if not psum_bufs:
        psum_bufs = (16 * 1024) // (psw * 4)  # fill all 8 banks
    n_lse = max(pat.count("L") for pat in patterns)
    n_g = max(pat.count("G") for pat in patterns)
    for pat in patterns:
        assert len(pat) == n_ps, pat
        assert all(c in "VABLG" for c in pat)
        assert pat[0] == "V", "first V initializes acc_v"

    nc = bacc.Bacc("TRN2", target_bir_lowering=False, debug=False)

    fa_dram = nc.dram_tensor(
        "fa", [P, KH, N + ROWS_PER_CORE], _FP8, kind="ExternalInput"
    )
    cls_dram = nc.dram_tensor(
        "cls", [ROW_TILES, P, 2, w_ship], _BF16, kind="ExternalOutput"
    )
    lse_dram = nc.dram_tensor(
        "lse", [ROW_TILES, P, max(n_lse, 1)], _F32, kind="ExternalOutput"
    )
    gcl_dram = nc.dram_tensor(
        "gcl", [ROW_TILES, P, 5, w_ship if n_g else 1], _BF16,
        kind="ExternalOutput",
    )

    with tile.TileContext(nc) as tc:
        with (
            tc.tile_pool(name="ft_pool", bufs=1) as ft_pool,
            tc.tile_pool(name="at_pool", bufs=1) as at_pool,
            tc.tile_pool(name="s_pool", bufs=8) as s_pool,
            tc.tile_pool(name="d_pool", bufs=4) as d_pool,
            tc.tile_pool(name="acc_pool", bufs=14) as acc_pool,
            tc.tile_pool(name="sc_pool", bufs=2) as sc_pool,
            tc.tile_pool(name="psum", bufs=psum_bufs, space="PSUM") as psum_pool,
        ):
            fa_sb = ft_pool.tile([P, KH, N + ROWS_PER_CORE], _FP8, tag="fa")
            # fused layout [at0 | ft | at_rest]: the first DMA carries both
            # operands of the first matmuls in one transfer (one HWDGE+sem)
            cuts = [0, P + 1024, P + 2048, P + 3072, P + 4096]
            blk = (N - 4096) // ft_split
            cuts += [P + 4096 + j * blk for j in range(1, ft_split)]
            cuts += [P + N, P + N + ROWS_PER_CORE - P]
            for j0, j1 in zip(cuts, cuts[1:]):
                nc.sync.dma_start(fa_sb[:, :, j0:j1], fa_dram[:, :, j0:j1])
            # view helpers
            def at_cols(r):
                return (
                    fa_sb[:, :, r * P : (r + 1) * P]
                    if r == 0
                    else fa_sb[:, :, P + N + (r - 1) * P : P + N + r * P]
                )

            def ft_chunk(c):
                return fa_sb[:, :, P + c * CHUNK : P + (c + 1) * CHUNK]

            dummies = [
                d_pool.tile([P, psw], _BF16, name=f"dummy{i}", tag="dummy")
                for i in range(4)
            ]


            for r in range(ROW_TILES):
                pattern = patterns[r % len(patterns)]
                accs = [
                    acc_pool.tile([P, psw], _BF16, name=f"acc{i}", tag="accv")
                    for i in range(2)
                ]
                scores = None
                if "L" in pattern:
                    scores = sc_pool.tile(
                        [P, max(n_lse, 1)], _F32, tag="sc"
                    )
                seen = [False, False]
                vi = 0
                li = 0
                acc_g = None
                gi = 0
                n_ag = min(pattern.count("G"), 5)
                if n_ag:
                    acc_g = [
                        acc_pool.tile([P, psw], _BF16, name=f"accg{i}", tag="accg")
                        for i in range(n_ag)
                    ]
                for t in range(n_ps):
                    ps = psum_pool.tile([P, psw], _F32, tag="ps")
                    for h in range(chunks_per_ps):
                        c = (t * psw) // CHUNK + h
                        nc.tensor.matmul(
                            ps[:, h * CHUNK : (h + 1) * CHUNK],
                            at_cols(r),
                            ft_chunk(c),
                            start=True,
                            stop=True,
                            perf_mode=mybir.MatmulPerfMode.DoubleRow,
                        )
                    ch = pattern[t]
                    if ch == "V":
                        idx = vi % 2
                        a = accs[idx]
                        vi += 1
                        if not seen[idx]:
                            nc.vector.tensor_scalar_max(a[:], ps[:], NEG)
                            seen[idx] = True
                        else:
                            nc.vector.tensor_tensor(a[:], ps[:], a[:], _MAX)
                    elif ch in "AB":
                        s = s_pool.tile([P, psw], _BF16, tag="sb")
                        nc.scalar.copy(s[:], ps[:])
                        idx = vi % 2
                        a = accs[idx]
                        vi += 1
                        if not seen[idx]:
                            raise AssertionError("fold before acc init")
                        nc.vector.tensor_tensor(a[:], s[:], a[:], _MAX)
                    elif ch == "L":
                        nc.scalar.activation(
                            dummies[li % 2][:],
                            ps[:],
                            mybir.ActivationFunctionType.Exp,
                            scale=LSE_SCALE,
                            accum_out=scores[:, li : li + 1],
                        )
                        li += 1
                    elif ch == "G":
                        a_g = acc_g[gi % n_ag]
                        if gi < n_ag:
                            # first two G copies initialize the G-accs
                            nc.scalar.activation(
                                a_g[:],
                                ps[:],
                                mybir.ActivationFunctionType.Exp,
                                scale=LSE_SCALE,
                            )
                        else:
                            sg = s_pool.tile([P, psw], _BF16, tag="sg")
                            nc.scalar.activation(
                                sg[:],
                                ps[:],
                                mybir.ActivationFunctionType.Exp,
                                scale=LSE_SCALE,
                            )
                            nc.gpsimd.tensor_tensor(
                                a_g[:], sg[:], a_g[:], _ADD
                            )
                        gi += 1
                assert seen[0] and seen[1] and w_ship == psw
                nc.sync.dma_start(cls_dram[r, :, 0, :], accs[0][:])
                nc.sync.dma_start(cls_dram[r, :, 1, :], accs[1][:])
                if acc_g is not None:
                    for i in range(min(gi, n_ag)):
                        nc.sync.dma_start(gcl_dram[r, :, i, :], acc_g[i][:])
                if n_lse and scores is not None:
                    nc.sync.dma_start(lse_dram[r], scores[:])

    nc.compile()
    return nc


_NC_CACHE = {}


def _get_nc():
    if "nc" not in _NC_CACHE:
        _NC_CACHE["nc"] = build_nc()
    return _NC_CACHE["nc"]


def make_inputs(feature: np.ndarray):
    """Host-side shard prep: F^T in [P, KH, cols] layout, fp8-quantized."""
    ft = np.ascontiguousarray(
        feature.T.reshape(KH, P, N).transpose(1, 0, 2)
    ).astype(_FP8_NP)
    in_maps = []
    for c in range(N_CORES):
        at = ft[:, :, c * ROWS_PER_CORE : (c + 1) * ROWS_PER_CORE]
        fa = np.concatenate([at[:, :, 0:P], ft, at[:, :, P:]], axis=2)
        in_maps.append({"fa": np.ascontiguousarray(fa)})
    return in_maps


def run_device(feature: np.ndarray, trace: bool = False):
    """Run the SPMD kernel.

    Returns (cls [N, W] f32 class maxima over non-L columns,
             lse [N, n_lse] f32 chunk scores, results)."""
    nc = _get_nc()
    in_maps = make_inputs(feature)
    res = run_bass_kernel_spmd(
        nc, in_maps, core_ids=list(range(N_CORES)), trace=trace
    )
    cls_parts, lse_parts, g_parts = [], [], []
    for r in res.results:
        c = np.asarray(r["cls"]).astype(np.float32)  # [RT, P, 2, W]
        cls_parts.append(c.max(axis=2).reshape(ROWS_PER_CORE, -1))
        lse_parts.append(np.asarray(r["lse"]).reshape(ROWS_PER_CORE, -1))
        g = np.asarray(r["gcl"]).astype(np.float32)  # [RT, P, 5, W]
        g_parts.append(g.sum(axis=2).reshape(ROWS_PER_CORE, -1))
    return (
        np.concatenate(cls_parts),
        np.concatenate(lse_parts),
        np.concatenate(g_parts),
        res,
    )


def recover_loss(
    feature: np.ndarray,
    cls: np.ndarray,
    lse: np.ndarray,
    gcl: np.ndarray | None = None,
) -> np.float32:
    """Exact argmax recovery + reference loss formula on host.

    cls[g] = per-class maxima (class = col mod W) over columns drained
    through the fold channels; lse[g] = per-L-chunk exp-sum scores for
    columns covered by LSE tiles. Candidates per row: top-K classes,
    the diagonal class, top-C LSE chunks, and the diagonal's own chunk
    if it lies in an LSE tile. All candidates evaluated in exact fp32.
    """
    n, w = feature.shape[0], cls.shape[1]
    B = n // w
    psw = PSW
    feat = np.ascontiguousarray(feature, dtype=np.float32)
    rows = np.arange(n)
    k = min(TOPK, w - 1)
    t_cls = np.argpartition(-cls, k, axis=1)[:, :k].astype(np.int64)

    best_val = np.full(n, -np.inf, dtype=np.float32)
    best_col = np.zeros(n, dtype=np.int64)

    def consider_cols(row_idx: np.ndarray, cols: np.ndarray):
        cd = feat[row_idx] @ feat[cols].T
        self_pos = np.searchsorted(cols, row_idx)
        kk = np.arange(len(row_idx))
        has_self = (self_pos < len(cols)) & (
            cols[np.minimum(self_pos, len(cols) - 1)] == row_idx
        )
        cd[kk[has_self], self_pos[has_self]] = -np.inf
        b = np.argmax(cd, axis=1)
        v = cd[kk, b]
        c = cols[b]
        upd = (v > best_val[row_idx]) | (
            (v == best_val[row_idx]) & (c < best_col[row_idx])
        )
        ri = row_idx[upd]
        best_val[ri] = v[upd]
        best_col[ri] = c[upd]

    # class candidates: top-K device classes (+ exp-domain G classes)
    # + diagonal class
    parts = [t_cls]
    if gcl is not None and gcl.shape[1] == w:
        kg = min(3, w - 1)
        parts.append(
            np.argpartition(-np.nan_to_num(gcl, nan=-np.inf), kg, axis=1)[
                :, :kg
            ].astype(np.int64)
        )
    parts.append((rows % w)[:, None])
    all_cls = np.concatenate(parts, axis=1)
    for j in range(all_cls.shape[1]):
        col = all_cls[:, j]
        order = np.argsort(col, kind="stable")
        bounds = np.searchsorted(col[order], np.arange(w + 1))
        for t in range(w):
            grp = order[bounds[t] : bounds[t + 1]]
            if len(grp):
                consider_cols(grp, t + w * np.arange(B))

    # LSE chunk candidates
    n_ps = N // psw
    lpos = {}  # row-tile index -> list of L tile positions
    for r in range(ROW_TILES):
        pat = PATTERNS[r % len(PATTERNS)]
        lpos[r] = [t for t in range(n_ps) if pat[t] == "L"]
    n_lse = lse.shape[1] if any("L" in p for p in PATTERNS) else 0
    if n_lse:
        rt = (rows % ROWS_PER_CORE) // P  # row-tile index per row
        lchunks = np.full((n, n_lse), -1, dtype=np.int64)
        for r in range(ROW_TILES):
            sel = rt == r
            for j, t in enumerate(lpos[r]):
                lchunks[sel, j] = t
        # slots without an L tile carry garbage scores; mask them out
        lse = np.where(
            lchunks >= 0, np.nan_to_num(lse, nan=-np.inf), -np.inf
        )
        cc = min(TOPC, n_lse)
        top = np.argpartition(-lse, cc - 1, axis=1)[:, :cc]
        want = np.zeros((n, n_ps), dtype=bool)
        kk = np.arange(n)[:, None]
        want[kk, lchunks[kk, top]] = True
        # diagonal chunk if the row's own column lies in an L tile
        diag_t = rows // psw % n_ps
        in_l = lchunks == diag_t[:, None]
        want[in_l.any(axis=1), diag_t[in_l.any(axis=1)]] = True
        for t in range(n_ps):
            grp = rows[want[:, t]]
            if len(grp):
                consider_cols(grp, np.arange(t * psw, (t + 1) * psw))

    I = best_col
    diff = feat - feat[I] + EPS
    dist = np.sqrt((diff * diff).sum(axis=1))
    loss = -np.mean(np.log(n * dist))
    return np.float32(loss)


def kernel(feature: np.ndarray) -> np.ndarray:
    feature = np.asarray(feature, dtype=np.float32)
    for attempt in range(3):
        try:
            cls, lse, gcl, _res = run_device(feature)
            break
        except Exception:
            # transient device/tunnel hiccups; rebuild and retry
            _NC_CACHE.clear()
            if attempt == 2:
                raise
    return np.asarray(recover_loss(feature, cls, lse, gcl), dtype=np.float32)


if __name__ == "__main__":
    rng = np.random.default_rng(0)
    feature = rng.standard_normal((N, D), dtype=np.float32)
    print("loss:", kernel(feature))


# revision 14
# speedup vs baseline: 1.0599x; 1.0013x over previous
"""Trainium2 kernel v3 for nn_Loss_26886495273741 (retrieval_knn).

reference:
    dots = feature @ feature.T          # [n, n], n=16384, d=256
    dots[diag] = -1
    I = argmax(dots, axis=1)
    loss = -mean(log(n * ||feature - feature[I] + 1e-6||_2))

Device strategy (8 NeuronCores, SPMD, host-replicated "all-gather"):
  * Rows sharded: core c owns rows [c*2048, (c+1)*2048).
  * fp8 DoubleRow matmuls fill PSUM tiles [128, 1024] fp32 (x4 bufs)
    with the row-block of dots, 16 tiles per 128-row tile.
  * Hardware constraints (verified): only ACT and DVE can read PSUM and
    only one PSUM operand per instruction, so the drain is split:
      V: DVE absorb  tensor_tensor max(ps, acc_v) -> acc_v   (bf16 acc)
      A: ACT copy -> bf16 s;  Pool folds s into acc_p
      B: ACT copy -> bf16 s;  DVE folds s into acc_v
      L: ACT exp(s*dot) with fp32 accumulator -> per-chunk LSE score
         (self-contained; those columns are covered by chunk scores,
         not by the class fold)
  * acc_v/acc_p are halved to W classes (DVE / Pool) and shipped; LSE
    chunk scores ship as fp32. Host: top-K classes + top LSE chunks +
    diagonal class/chunk, exact fp32 candidate eval, reference loss.
"""

import os
import sys

import numpy as np

_jp = os.environ.get("JAX_PLATFORMS")
if _jp is not None and "axon" not in _jp:
    os.environ["JAX_PLATFORMS"] = "axon," + _jp

try:
    import concourse.bass as bass  # noqa: F401
except ImportError:  # grading env runs from a bare directory
    sys.path.insert(0, "/opt/trn_rl_repo")

import ml_dtypes  # noqa: F401

import concourse.bass as bass
import concourse.mybir as mybir
import concourse.tile as tile
from concourse import bacc
from concourse.bass_utils import run_bass_kernel_spmd

# Problem geometry (hardcoded per spec.json: feature [16384, 256] f32).
N = 16384
D = 256
N_CORES = 8
ROWS_PER_CORE = N // N_CORES  # 2048
P = 128
ROW_TILES = ROWS_PER_CORE // P  # 16
CHUNK = 512  # matmul free dim == one PSUM bank (fp32)
KH = D // P  # 2 contraction halves packed for DoubleRow

EPS = 1e-6

_BF16 = mybir.dt.bfloat16
_F32 = mybir.dt.float32
_FP8 = mybir.dt.float8e4
_FP8_NP = mybir.dt.np(_FP8)

_MAX = mybir.AluOpType.max
_ADD = mybir.AluOpType.add

# --- tunables (swept offline with TimelineSim; best kept hardcoded) ---
PSW = 1024  # psum tile width (2 banks, 4 bufs)
# per-row-tile channel patterns, cycled over row tiles.
# V=DVE absorb, A=ACT copy+Pool fold, B=ACT copy+DVE fold, L=ACT LSE
PATTERNS = (
    ("VGVGVGVGVGVGVGVG",)
    + ("VGVGVGVGVGVGVGVG", "VGVGVGVGVGVGVGGG") * 7
    + ("VGVLVLVGVLVLVLVL",)
)
W_SHIP = 1024  # classes shipped per row (= PSW: no halving)
FT_SPLIT = 12  # column blocks for the big ft load
TOPK = 4  # host-side top-k classes
TOPC = 3  # host-side top-k LSE chunks
LSE_SCALE = 0.275  # exp scale for LSE scores

NEG = -3.0e38


def build_nc(
    psw: int = PSW,
    patterns: tuple = PATTERNS,
    w_ship: int = W_SHIP,
    ft_split: int = FT_SPLIT,
    psum_bufs: int = 0,
):
    n_ps = N // psw  # psum tiles per row-tile
    chunks_per_ps = psw // CHUNK
    if not psum_bufs:
        psum_bufs = (16 * 1024) // (psw * 4)  # fill all 8 banks
    n_lse = max(pat.count("L") for pat in patterns)
    n_g = max(pat.count("G") for pat in patterns)
    for pat in patterns:
        assert len(pat) == n_ps, pat
        assert all(c in "VABLG" for c in pat)
        assert pat[0] == "V", "first V initializes acc_v"

    nc = bacc.Bacc("TRN2", target_bir_lowering=False, debug=False)

    fa_dram = nc.dram_tensor(
        "fa", [P, KH, N + ROWS_PER_CORE], _FP8, kind="ExternalInput"
    )
    cls_dram = nc.dram_tensor(
        "cls", [ROW_TILES, P, 2, w_ship], _BF16, kind="ExternalOutput"
    )
    lse_dram = nc.dram_tensor(
        "lse", [ROW_TILES, P, max(n_lse, 1)], _F32, kind="ExternalOutput"
    )
    gcl_dram = nc.dram_tensor(
        "gcl", [ROW_TILES, P, 5, w_ship if n_g else 1], _BF16,
        kind="ExternalOutput",
    )

    with tile.TileContext(nc) as tc:
        with (
            tc.tile_pool(name="ft_pool", bufs=1) as ft_pool,
            tc.tile_pool(name="at_pool", bufs=1) as at_pool,
            tc.tile_pool(name="s_pool", bufs=8) as s_pool,
            tc.tile_pool(name="d_pool", bufs=4) as d_pool,
            tc.tile_pool(name="acc_pool", bufs=14) as acc_pool,
            tc.tile_pool(name="sc_pool", bufs=2) as sc_pool,
            tc.tile_pool(name="psum", bufs=psum_bufs, space="PSUM") as psum_pool,
        ):
            fa_sb = ft_pool.tile([P, KH, N + ROWS_PER_CORE], _FP8, tag="fa")
            # fused layout [at0 | ft | at_rest]: the first DMA carries both
            # operands of the first matmuls in one transfer (one HWDGE+sem)
            cuts = [0, P + 1024, P + 2048, P + 3072, P + 4096]
            blk = (N - 4096) // ft_split
            cuts += [P + 4096 + j * blk for j in range(1, ft_split)]
            cuts += [P + N, P + N + ROWS_PER_CORE - P]
            for j0, j1 in zip(cuts, cuts[1:]):
                nc.sync.dma_start(fa_sb[:, :, j0:j1], fa_dram[:, :, j0:j1])
            # view helpers
            def at_cols(r):
                return (
                    fa_sb[:, :, r * P : (r + 1) * P]
                    if r == 0
                    else fa_sb[:, :, P + N + (r - 1) * P : P + N + r * P]
                )

            def ft_chunk(c):
                return fa_sb[:, :, P + c * CHUNK : P + (c + 1) * CHUNK]

            dummies = [
                d_pool.tile([P, psw], _BF16, name=f"dummy{i}", tag="dummy")
                for i in range(4)
            ]


            for r in range(ROW_TILES):
                pattern = patterns[r % len(patterns)]
                accs = [
                    acc_pool.tile([P, psw], _BF16, name=f"acc{i}", tag="accv")
                    for i in range(2)
                ]
                scores = None
                if "L" in pattern:
                    scores = sc_pool.tile(
                        [P, max(n_lse, 1)], _F32, tag="sc"
                    )
                seen = [False, False]
                vi = 0
                li = 0
                acc_g = None
                gi = 0
                n_ag = min(pattern.count("G"), 5)
                if n_ag:
                    acc_g = [
                        acc_pool.tile([P, psw], _BF16, name=f"accg{i}", tag="accg")
                        for i in range(n_ag)
                    ]
                for t in range(n_ps):
                    ps = psum_pool.tile([P, psw], _F32, tag="ps")
                    for h in range(chunks_per_ps):
                        c = (t * psw) // CHUNK + h
                        nc.tensor.matmul(
                            ps[:, h * CHUNK : (h + 1) * CHUNK],
                            at_cols(r),
                            ft_chunk(c),
                            start=True,
                            stop=True,
                            perf_mode=mybir.MatmulPerfMode.DoubleRow,
                        )
                    ch = pattern[t]
                    if ch == "V":
                        idx = vi % 2
                        a = accs[idx]
                        vi += 1
                        if not seen[idx]:
                            nc.vector.tensor_scalar_max(a[:], ps[:], NEG)
                            seen[idx] = True
                        else:
                            nc.vector.tensor_tensor(a[:], ps[:], a[:], _MAX)
                    elif ch in "AB":
                        s = s_pool.tile([P, psw], _BF16, tag="sb")
                        nc.scalar.copy(s[:], ps[:])
                        idx = vi % 2
                        a = accs[idx]
                        vi += 1
                        if not seen[idx]:
                            raise AssertionError("fold before acc init")
                        nc.vector.tensor_tensor(a[:], s[:], a[:], _MAX)
                    elif ch == "L":
                        nc.scalar.activation(
                            dummies[li % 2][:],
                            ps[:],
                            mybir.ActivationFunctionType.Exp,
                            scale=LSE_SCALE,
                            accum_out=scores[:, li : li + 1],
                        )
                        li += 1
                    elif ch == "G":
                        a_g = acc_g[gi % n_ag]
                        if gi < n_ag:
                            # first two G copies initialize the G-accs
                            nc.scalar.activation(
                                a_g[:],
                                ps[:],
                                mybir.ActivationFunctionType.Exp,
                                scale=LSE_SCALE,
                            )
                        else:
                            sg = s_pool.tile([P, psw], _BF16, tag="sg")
                            nc.scalar.activation(
                                sg[:],
                                ps[:],
                                mybir.ActivationFunctionType.Exp,
                                scale=LSE_SCALE,
                            )
                            nc.gpsimd.tensor_tensor(
                                a_g[:], sg[:], a_g[:], _ADD
                            )
                        gi += 1
                assert seen[0] and seen[1] and w_ship == psw
                nc.sync.dma_start(cls_dram[r, :, 0, :], accs[0][:])
                nc.sync.dma_start(cls_dram[r, :, 1, :], accs[1][:])
                if acc_g is not None:
                    for i in range(min(gi, n_ag)):
                        nc.sync.dma_start(gcl_dram[r, :, i, :], acc_g[i][:])
                if n_lse and scores is not None:
                    nc.sync.dma_start(lse_dram[r], scores[:])

    nc.compile()
    return nc


_NC_CACHE = {}


def _get_nc():
    if "nc" not in _NC_CACHE:
        _NC_CACHE["nc"] = build_nc()
    return _NC_CACHE["nc"]


def make_inputs(feature: np.ndarray):
    """Host-side shard prep: F^T in [P, KH, cols] layout, fp8-quantized."""
    ft = np.ascontiguousarray(
        feature.T.reshape(KH, P, N).transpose(1, 0, 2)
    ).astype(_FP8_NP)
    in_maps = []
    for c in range(N_CORES):
        at = ft[:, :, c * ROWS_PER_CORE : (c + 1) * ROWS_PER_CORE]
        fa = np.concatenate([at[:, :, 0:P], ft, at[:, :, P:]], axis=2)
        in_maps.append({"fa": np.ascontiguousarray(fa)})
    return in_maps


def run_device(feature: np.ndarray, trace: bool = False):
    """Run the SPMD kernel.

    Returns (cls [N, W] f32 class maxima over non-L columns,
             lse [N, n_lse] f32 chunk scores, results)."""
    nc = _get_nc()
    in_maps = make_inputs(feature)
    res = run_bass_kernel_spmd(
        nc, in_maps, core_ids=list(range(N_CORES)), trace=trace
    )
    cls_parts, lse_parts, g_parts = [], [], []
    for r in res.results:
        c = np.asarray(r["cls"]).astype(np.float32)  # [RT, P, 2, W]
        cls_parts.append(c.max(axis=2).reshape(ROWS_PER_CORE, -1))
        lse_parts.append(np.asarray(r["lse"]).reshape(ROWS_PER_CORE, -1))
        g = np.asarray(r["gcl"]).astype(np.float32)  # [RT, P, 5, W]
        g_parts.append(g.sum(axis=2).reshape(ROWS_PER_CORE, -1))
    return (
        np.concatenate(cls_parts),
        np.concatenate(lse_parts),
        np.concatenate(g_parts),
        res,
    )


def recover_loss(
    feature: np.ndarray,
    cls: np.ndarray,
    lse: np.ndarray,
    gcl: np.ndarray | None = None,
) -> np.float32:
    """Exact argmax recovery + reference loss formula on host.

    cls[g] = per-class maxima (class = col mod W) over columns drained
    through the fold channels; lse[g] = per-L-chunk exp-sum scores for
    columns covered by LSE tiles. Candidates per row: top-K classes,
    the diagonal class, top-C LSE chunks, and the diagonal's own chunk
    if it lies in an LSE tile. All candidates evaluated in exact fp32.
    """
    n, w = feature.shape[0], cls.shape[1]
    B = n // w
    psw = PSW
    feat = np.ascontiguousarray(feature, dtype=np.float32)
    rows = np.arange(n)
    k = min(TOPK, w - 1)
    t_cls = np.argpartition(-cls, k, axis=1)[:, :k].astype(np.int64)

    best_val = np.full(n, -np.inf, dtype=np.float32)
    best_col = np.zeros(n, dtype=np.int64)

    def consider_cols(row_idx: np.ndarray, cols: np.ndarray):
        cd = feat[row_idx] @ feat[cols].T
        self_pos = np.searchsorted(cols, row_idx)
        kk = np.arange(len(row_idx))
        has_self = (self_pos < len(cols)) & (
            cols[np.minimum(self_pos, len(cols) - 1)] == row_idx
        )
        cd[kk[has_self], self_pos[has_self]] = -np.inf
        b = np.argmax(cd, axis=1)
        v = cd[kk, b]
        c = cols[b]
        upd = (v > best_val[row_idx]) | (
            (v == best_val[row_idx]) & (c < best_col[row_idx])
        )
        ri = row_idx[upd]
        best_val[ri] = v[upd]
        best_col[ri] = c[upd]

    # class candidates: top-K device classes (+ exp-domain G classes)
    # + diagonal class
    parts = [t_cls]
    if gcl is not None and gcl.shape[1] == w:
        kg = min(3, w - 1)
        parts.append(
            np.argpartition(-np.nan_to_num(gcl, nan=-np.inf), kg, axis=1)[
                :, :kg
            ].astype(np.int64)
        )
    parts.append((rows % w)[:, None])
    all_cls = np.concatenate(parts, axis=1)
    for j in range(all_cls.shape[1]):
        col = all_cls[:, j]
        order = np.argsort(col, kind="stable")
        bounds = np.searchsorted(col[order], np.arange(w + 1))
        for t in range(w):
            grp = order[bounds[t] : bounds[t + 1]]
            if len(grp):
                consider_cols(grp, t + w * np.arange(B))

    # LSE chunk candidates
    n_ps = N // psw
    lpos = {}  # row-tile index -> list of L tile positions
    for r in range(ROW_TILES):
        pat = PATTERNS[r % len(PATTERNS)]
        lpos[r] = [t for t in range(n_ps) if pat[t] == "L"]
    n_lse = lse.shape[1] if any("L" in p for p in PATTERNS) else 0
    if n_lse:
        rt = (rows % ROWS_PER_CORE) // P  # row-tile index per row
        lchunks = np.full((n, n_lse), -1, dtype=np.int64)
        for r in range(ROW_TILES):
            sel = rt == r
            for j, t in enumerate(lpos[r]):
                lchunks[sel, j] = t
        # slots without an L tile carry garbage scores; mask them out
        lse = np.where(
            lchunks >= 0, np.nan_to_num(lse, nan=-np.inf), -np.inf
        )
        cc = min(TOPC, n_lse)
        top = np.argpartition(-lse, cc - 1, axis=1)[:, :cc]
        want = np.zeros((n, n_ps), dtype=bool)
        kk = np.arange(n)[:, None]
        want[kk, lchunks[kk, top]] = True
        # diagonal chunk if the row's own column lies in an L tile
        diag_t = rows // psw % n_ps
        in_l = lchunks == diag_t[:, None]
        want[in_l.any(axis=1), diag_t[in_l.any(axis=1)]] = True
        for t in range(n_ps):
            grp = rows[want[:, t]]
            if len(grp):
                consider_cols(grp, np.arange(t * psw, (t + 1) * psw))

    I = best_col
    diff = feat - feat[I] + EPS
    dist = np.sqrt((diff * diff).sum(axis=1))
    loss = -np.mean(np.log(n * dist))
    return np.float32(loss)


def kernel(feature: np.ndarray) -> np.ndarray:
    feature = np.asarray(feature, dtype=np.float32)
    for attempt in range(3):
        try:
            cls, lse, gcl, _res = run_device(feature)
            break
        except Exception:
            # transient device/tunnel hiccups; rebuild and retry
            _NC_CACHE.clear()
            if attempt == 2:
                raise
    return np.asarray(recover_loss(feature, cls, lse, gcl), dtype=np.float32)


if __name__ == "__main__":
    rng = np.random.default_rng(0)
    feature = rng.standard_normal((N, D), dtype=np.float32)
    print("loss:", kernel(feature))
